# revision 27
# baseline (speedup 1.0000x reference)
"""BSpline activation (KAN-style) forward on 8 NeuronCores.

Math: reference computes out[b,n,j] = sum_{i,k} B_k(x[b,n,i]) * W[k,i,j]
where B_k are cubic B-spline bases on a uniform grid (spacing 0.4, range
[-2.2, 2.2]) and x is uniform in [0,1).  On [0,1) the 8 bases live in the
6-dim space of C^2 piecewise cubics with interior knots {0.2, 0.6}, so
    B_k(x) = A[0,k]*1 + A[1,k]*x + A[2,k]*x^2 + A[3,k]*x^3
           + A[4,k]*relu(x-0.2)^3 + A[5,k]*relu(x-0.6)^3      (exact)
Folding A into W gives out = bias + Phi(x) @ V with a 5-feature contraction
of size 5*256 = 1280 per output element - a dense matmul on TensorE, with
the pointwise features Phi computed on ACT (squares) + DVE (fused cubes).

Sharding: data-parallel over the 16384 (b,n) rows -> 2048 rows/core.
Per core: x^T [256, 2048] in, y^T [256, 2048] out (transposes on host).
"""

import numpy as np

_COMPILED = None  # (nc, meta) cache

# ---------------------------------------------------------------- host math

SPLINE_ORDER = 3


def _spline_bases_np(x, g, order):
    # Cox-de Boor, float64, mirrors the reference implementation.
    gg = g.reshape((-1,) + (1,) * x.ndim)
    bases = ((x >= gg[:-1]) & (x < gg[1:])).astype(x.dtype)
    for k in range(1, order + 1):
        b1 = (x - gg[:-(k + 1)]) / (gg[k:-1] - gg[:-(k + 1)]) * bases[:-1]
        b2 = (gg[k + 1:] - x) / (gg[k + 1:] - gg[1:-k]) * bases[1:]
        bases = b1 + b2
    return np.moveaxis(bases, 0, -1)  # [..., K]


def _solve_A(grid):
    """A [6, 8] with B_k(x) = sum_f A[f,k] * phi_f(x) exactly on [0,1).

    phi = [1, x, (x-k1)^2, (x-k1)^3, relu(x-k1)^3, relu(x-k2)^3] - chosen so
    the device computes each non-constant feature in at most 2 cheap ops.
    """
    g = np.asarray(grid, np.float64)
    kn = g[(g > 1e-9) & (g < 1.0 - 1e-9)]  # interior knots in (0,1): [0.2, 0.6]
    assert kn.shape == (2,), kn
    xs = np.linspace(0.0, 1.0, 4001, endpoint=False)
    B = _spline_bases_np(xs, g, SPLINE_ORDER)  # [S, 8]
    t1 = xs - kn[0]
    r1 = np.maximum(t1, 0.0)
    r2 = np.maximum(xs - kn[1], 0.0)
    P = np.stack([np.ones_like(xs), xs, t1 * t1, t1**3, r1**3, r2**3], -1)
    A, *_ = np.linalg.lstsq(P, B, rcond=None)  # [6, 8]
    recon = P @ A
    assert np.abs(recon - B).max() < 1e-10
    return A, float(kn[0]), float(kn[1])


# ------------------------------------------------------------- device kernel

NCORES = 8
ROWS = 2048          # (b,n) rows per core
CIN = 256            # in channels
COUT = 256           # out channels
NF = 5               # non-constant features: x, x^2, x^3, r1^3, r2^3
KCH = NF * 2         # 128-partition contraction chunks (2 per feature)
BT = 4               # bn tiles of 512
TOK = ROWS // BT     # 512


def _build(k1, k2):
    """Build + compile the SPMD Bass program (same on all 8 cores)."""
    import concourse.bacc as bacc
    import concourse.tile as tile
    from concourse import mybir

    AF = mybir.ActivationFunctionType
    ALU = mybir.AluOpType
    fp = mybir.dt.float32
    fpr = mybir.dt.float32r

    nc = bacc.Bacc(
        "TRN2", target_bir_lowering=False, debug=False, num_devices=NCORES
    )
    x_t = nc.dram_tensor("x_t", [CIN, ROWS], fpr, kind="ExternalInput").ap()
    w = nc.dram_tensor("w", [KCH * 128, COUT], fpr, kind="ExternalInput").ap()
    bias = nc.dram_tensor("bias", [128, 2], fp, kind="ExternalInput").ap()
    y_t = nc.dram_tensor("y_t", [COUT, ROWS], fp, kind="ExternalOutput").ap()

    with tile.TileContext(nc) as tc:
        from contextlib import ExitStack

        with ExitStack() as ctx:
            cpool = ctx.enter_context(tc.tile_pool(name="const", bufs=1))
            xpool = ctx.enter_context(tc.tile_pool(name="x", bufs=1))
            fpool = ctx.enter_context(tc.tile_pool(name="feat", bufs=1))
            spool = ctx.enter_context(tc.tile_pool(name="scratch", bufs=1))
            wpool = ctx.enter_context(tc.tile_pool(name="w", bufs=1))
            ppool = ctx.enter_context(tc.tile_pool(name="ps", bufs=1, space="PSUM"))
            opool = ctx.enter_context(tc.tile_pool(name="out", bufs=4))

            # --- input DMAs on the two fast HWDGE queues (sync/scalar),
            # interleaved in consumption order; x in halves for early start ---
            HTOK = ROWS // 2  # 1024
            negk1 = cpool.tile([128, 1], fp)
            nc.gpsimd.memset(negk1[:], -k1)
            negk2 = cpool.tile([128, 1], fp)
            nc.gpsimd.memset(negk2[:], -k2)

            xs = [
                xpool.tile([128, ROWS], fpr, tag=f"x{h}", name=f"xt{h}")
                for h in range(2)
            ]
            wt = [
                wpool.tile([128, COUT], fpr, tag=f"w{k}", name=f"wt{k}")
                for k in range(KCH)
            ]
            bias_t = cpool.tile([128, 2], fp)

            def xdma(eng, h, c):
                eng.dma_start(
                    xs[h][:, c * HTOK:(c + 1) * HTOK],
                    x_t[h * 128:(h + 1) * 128, c * HTOK:(c + 1) * HTOK],
                )

            def wdma(eng, k):
                eng.dma_start(wt[k][:], w[k * 128:(k + 1) * 128, :])

            # x monopolizes the two fast HWDGE queues (sync + scalar; the
            # HWDGE round-robins among queued transfers, so anything else
            # queued with x delays x). w0 is small and needed first -> SWDGE.
            # Remaining w tiles stream behind: early ones on HWDGE after x,
            # late-deadline ones on SWDGE.
            wdma(nc.sync, 0)      # small, first MM needs it
            wdma(nc.scalar, 1)
            xdma(nc.sync, 0, 0)   # x0a
            xdma(nc.scalar, 0, 1)  # x0b
            xdma(nc.scalar, 1, 0)  # x1a
            xdma(nc.sync, 1, 1)   # x1b
            nc.gpsimd.dma_start(bias_t[:], bias[:])
            for k in range(2, 8):
                wdma(nc.sync if k % 2 == 0 else nc.scalar, k)
            for k in range(8, KCH):
                wdma(nc.gpsimd, k)  # late deadlines tolerate the slow SWDGE

            # --- features per in-channel half h:
            # [x, q1=(x-k1)^2, c1=(x-k1)^3, r1c=relu(c1), r2c=relu((x-k2)^3)]
            # computed per free-dim half-chunk c for pipelining ---
            def ftile(nm, h):
                return fpool.tile([128, ROWS], fpr, tag=f"{nm}_{h}", name=f"{nm}_{h}")

            def stile(nm, h):
                return spool.tile([128, ROWS], fp, tag=f"{nm}_{h}", name=f"s{nm}_{h}")

            q1 = [ftile("q1", h) for h in range(2)]
            c1 = [ftile("c1", h) for h in range(2)]
            r1c = [ftile("r1c", h) for h in range(2)]
            r2c = [ftile("r2c", h) for h in range(2)]
            q2 = [stile("q2", h) for h in range(2)]
            c2 = [stile("c2", h) for h in range(2)]

            for h in range(2):
                for c in range(2):
                    sl = slice(c * HTOK, (c + 1) * HTOK)
                    x_ = xs[h][:, sl]
                    # ACT: squares directly from x (bias folds the shift)
                    nc.scalar.activation(q1[h][:, sl], x_, AF.Square, bias=negk1[:])
                    nc.scalar.activation(q2[h][:, sl], x_, AF.Square, bias=negk2[:])
                    # DVE: signed cubes via fused (x - k) * q
                    nc.vector.scalar_tensor_tensor(
                        c1[h][:, sl], x_, -k1, q1[h][:, sl], ALU.add, ALU.mult
                    )
                    nc.vector.scalar_tensor_tensor(
                        c2[h][:, sl], x_, -k2, q2[h][:, sl], ALU.add, ALU.mult
                    )
                    # truncate: relu(c) == relu(x-k)^3 elementwise
                    nc.scalar.activation(r1c[h][:, sl], c1[h][:, sl], AF.Relu)
                    nc.vector.tensor_scalar_max(r2c[h][:, sl], c2[h][:, sl], 0.0)

            feats = [xs, q1, c1, r1c, r2c]  # [f][h]

            ps = [
                [
                    ppool.tile(
                        [128, TOK], fp, tag=f"ps{oc}_{bt}", name=f"ps{oc}_{bt}"
                    )
                    for bt in range(BT)
                ]
                for oc in range(2)
            ]
            for ki in range(KCH):
                f, h = ki // 2, ki % 2
                rhs_full = feats[f][h]
                for oc in range(2):
                    lhsT = wt[ki][:, oc * 128:(oc + 1) * 128]
                    for bt in range(BT):
                        nc.tensor.matmul(
                            ps[oc][bt][:, :],
                            lhsT=lhsT,
                            rhs=rhs_full[:, bt * TOK:(bt + 1) * TOK],
                            start=(ki == 0),
                            stop=(ki == KCH - 1),
                        )
                        if ki == KCH - 1:
                            # evict this bank right after its last matmul;
                            # alternate ACT/DVE to split the tail work
                            ot = opool.tile(
                                [128, TOK], fp, tag=f"o{bt % 2}", name=f"o{oc}_{bt}"
                            )
                            if bt % 2 == 0:
                                nc.scalar.activation(
                                    ot[:],
                                    ps[oc][bt][:],
                                    AF.Identity,
                                    bias=bias_t[:, oc:oc + 1],
                                )
                            else:
                                nc.vector.tensor_scalar_add(
                                    ot[:], ps[oc][bt][:], bias_t[:, oc:oc + 1]
                                )
                            (nc.sync if bt % 2 == 0 else nc.scalar).dma_start(
                                y_t[
                                    oc * 128:(oc + 1) * 128,
                                    bt * TOK:(bt + 1) * TOK,
                                ],
                                ot[:],
                            )

    nc.compile()
    return nc


def _round_fp32r(a):
    """Round fp32 array to fp32r precision (8e11m: keep top 20 bits, RNE)."""
    u = np.ascontiguousarray(a, np.float32).view(np.uint32).astype(np.uint64)
    u = (u + 0x7FF + ((u >> 12) & 1)) & 0xFFFFF000
    return u.astype(np.uint32).view(np.float32)


def _prepare(x, spline_kernel, grid):
    A, k1, k2 = _solve_A(grid)
    W = np.asarray(spline_kernel, np.float64)  # [8, 256, 256]
    V = np.einsum("fk,kij->fij", A, W)  # [6, 256, 256]
    bias = V[0].sum(axis=0)  # [256]
    wf = _round_fp32r(
        V[1:].reshape(NF, 2, 128, COUT).reshape(KCH * 128, COUT).astype(np.float32)
    )
    bias_t = np.ascontiguousarray(bias.reshape(2, 128).T, dtype=np.float32)
    xf = np.asarray(x, np.float32).reshape(NCORES, ROWS, CIN)
    x_shards = _round_fp32r(xf.transpose(0, 2, 1))  # [8, 256, 2048]
    return x_shards, wf, bias_t, k1, k2


def _get_compiled(k1, k2):
    global _COMPILED
    if _COMPILED is None:
        _COMPILED = _build(k1, k2)
    return _COMPILED


_LDW_PATCHED = False


def _maybe_patch_ldw_opt():
    """Optionally flip walrus --enable-ldw-opt (dedupes repeated LDWEIGHTS)."""
    global _LDW_PATCHED
    import os

    if _LDW_PATCHED or os.environ.get("BSPLINE_LDW_OPT") == "0":
        return
    import concourse.bass_utils as bu

    orig = bu.run_command

    def patched(argv, **kw):
        argv = [
            a.replace("--enable-ldw-opt=false", "--enable-ldw-opt=true")
            for a in argv
        ]
        return orig(argv, **kw)

    bu.run_command = patched
    _LDW_PATCHED = True


def kernel(x, spline_kernel, grid, _trace=False):
    from concourse.bass_utils import run_bass_kernel_spmd

    _maybe_patch_ldw_opt()

    x_shards, wf, bias_t, k1, k2 = _prepare(x, spline_kernel, grid)
    nc = _get_compiled(k1, k2)
    in_maps = [
        {"x_t": x_shards[c], "w": wf, "bias": bias_t} for c in range(NCORES)
    ]
    res = run_bass_kernel_spmd(
        nc, in_maps, list(range(NCORES)), trace=_trace
    )
    y = np.stack([res.results[c]["y_t"].T for c in range(NCORES)])
    out = np.ascontiguousarray(y, dtype=np.float32).reshape(x.shape[0], x.shape[1], COUT)
    if _trace:
        kernel._last_results = res
    return out


# revision 28
# speedup vs baseline: 1.0077x; 1.0077x over previous
"""BSpline activation (KAN-style) forward on 8 NeuronCores.

Math: reference computes out[b,n,j] = sum_{i,k} B_k(x[b,n,i]) * W[k,i,j]
where B_k are cubic B-spline bases on a uniform grid (spacing 0.4, range
[-2.2, 2.2]) and x is uniform in [0,1).  On [0,1) the 8 bases live in the
6-dim space of C^2 piecewise cubics with interior knots {0.2, 0.6}, so
    B_k(x) = A[0,k]*1 + A[1,k]*x + A[2,k]*x^2 + A[3,k]*x^3
           + A[4,k]*relu(x-0.2)^3 + A[5,k]*relu(x-0.6)^3      (exact)
Folding A into W gives out = bias + Phi(x) @ V with a 5-feature contraction
of size 5*256 = 1280 per output element - a dense matmul on TensorE, with
the pointwise features Phi computed on ACT (squares) + DVE (fused cubes).

Sharding: data-parallel over the 16384 (b,n) rows -> 2048 rows/core.
Per core: x^T [256, 2048] in, y^T [256, 2048] out (transposes on host).
"""

import numpy as np

_COMPILED = None  # (nc, meta) cache

# ---------------------------------------------------------------- host math

SPLINE_ORDER = 3


def _spline_bases_np(x, g, order):
    # Cox-de Boor, float64, mirrors the reference implementation.
    gg = g.reshape((-1,) + (1,) * x.ndim)
    bases = ((x >= gg[:-1]) & (x < gg[1:])).astype(x.dtype)
    for k in range(1, order + 1):
        b1 = (x - gg[:-(k + 1)]) / (gg[k:-1] - gg[:-(k + 1)]) * bases[:-1]
        b2 = (gg[k + 1:] - x) / (gg[k + 1:] - gg[1:-k]) * bases[1:]
        bases = b1 + b2
    return np.moveaxis(bases, 0, -1)  # [..., K]


def _solve_A(grid):
    """A [6, 8] with B_k(x) = sum_f A[f,k] * phi_f(x) exactly on [0,1).

    phi = [1, x, (x-k1)^2, (x-k1)^3, relu(x-k1)^3, relu(x-k2)^3] - chosen so
    the device computes each non-constant feature in at most 2 cheap ops.
    """
    g = np.asarray(grid, np.float64)
    kn = g[(g > 1e-9) & (g < 1.0 - 1e-9)]  # interior knots in (0,1): [0.2, 0.6]
    assert kn.shape == (2,), kn
    xs = np.linspace(0.0, 1.0, 4001, endpoint=False)
    B = _spline_bases_np(xs, g, SPLINE_ORDER)  # [S, 8]
    t1 = xs - kn[0]
    r1 = np.maximum(t1, 0.0)
    r2 = np.maximum(xs - kn[1], 0.0)
    P = np.stack([np.ones_like(xs), xs, t1 * t1, t1**3, r1**3, r2**3], -1)
    A, *_ = np.linalg.lstsq(P, B, rcond=None)  # [6, 8]
    recon = P @ A
    assert np.abs(recon - B).max() < 1e-10
    return A, float(kn[0]), float(kn[1])


# ------------------------------------------------------------- device kernel

NCORES = 8
ROWS = 2048          # (b,n) rows per core
CIN = 256            # in channels
COUT = 256           # out channels
NF = 5               # non-constant features: x, x^2, x^3, r1^3, r2^3
KCH = NF * 2         # 128-partition contraction chunks (2 per feature)
BT = 4               # bn tiles of 512
TOK = ROWS // BT     # 512


def _build(k1, k2):
    """Build + compile the SPMD Bass program (same on all 8 cores)."""
    import concourse.bacc as bacc
    import concourse.tile as tile
    from concourse import mybir

    AF = mybir.ActivationFunctionType
    ALU = mybir.AluOpType
    fp = mybir.dt.float32
    fpr = mybir.dt.float32r

    nc = bacc.Bacc(
        "TRN2", target_bir_lowering=False, debug=False, num_devices=NCORES
    )
    x_t = nc.dram_tensor("x_t", [CIN, ROWS], fpr, kind="ExternalInput").ap()
    w = nc.dram_tensor("w", [KCH * 128, COUT], fpr, kind="ExternalInput").ap()
    bias = nc.dram_tensor("bias", [128, 2], fp, kind="ExternalInput").ap()
    y_t = nc.dram_tensor("y_t", [COUT, ROWS], fp, kind="ExternalOutput").ap()

    with tile.TileContext(nc) as tc:
        from contextlib import ExitStack

        with ExitStack() as ctx:
            cpool = ctx.enter_context(tc.tile_pool(name="const", bufs=1))
            xpool = ctx.enter_context(tc.tile_pool(name="x", bufs=1))
            fpool = ctx.enter_context(tc.tile_pool(name="feat", bufs=1))
            spool = ctx.enter_context(tc.tile_pool(name="scratch", bufs=1))
            wpool = ctx.enter_context(tc.tile_pool(name="w", bufs=1))
            ppool = ctx.enter_context(tc.tile_pool(name="ps", bufs=1, space="PSUM"))
            opool = ctx.enter_context(tc.tile_pool(name="out", bufs=4))

            # --- input DMAs on the two fast HWDGE queues (sync/scalar),
            # interleaved in consumption order; x in halves for early start ---
            HTOK = ROWS // 2  # 1024
            negk1 = cpool.tile([128, 1], fp)
            nc.gpsimd.memset(negk1[:], -k1)
            negk2 = cpool.tile([128, 1], fp)
            nc.gpsimd.memset(negk2[:], -k2)

            xs = [
                xpool.tile([128, ROWS], fpr, tag=f"x{h}", name=f"xt{h}")
                for h in range(2)
            ]
            wt = [
                wpool.tile([128, COUT], fpr, tag=f"w{k}", name=f"wt{k}")
                for k in range(KCH)
            ]
            bias_t = cpool.tile([128, 2], fp)

            def xdma(eng, h, c):
                eng.dma_start(
                    xs[h][:, c * HTOK:(c + 1) * HTOK],
                    x_t[h * 128:(h + 1) * 128, c * HTOK:(c + 1) * HTOK],
                )

            def wdma(eng, k):
                eng.dma_start(wt[k][:], w[k * 128:(k + 1) * 128, :])

            # x monopolizes the two fast HWDGE queues (sync + scalar; the
            # HWDGE round-robins among queued transfers, so anything else
            # queued with x delays x). w0 is small and needed first -> SWDGE.
            # Remaining w tiles stream behind: early ones on HWDGE after x,
            # late-deadline ones on SWDGE.
            # scalar engine gets at most 3 triggers (a 4th+ can block ACT
            # compute behind queue-full waits); sync (idle engine) takes the
            # rest; bias + last w tiles ride the slow SWDGE (late deadlines).
            wdma(nc.sync, 0)      # small, first MM needs it
            wdma(nc.scalar, 1)
            xdma(nc.sync, 0, 0)   # x0a
            xdma(nc.scalar, 0, 1)  # x0b
            xdma(nc.scalar, 1, 0)  # x1a
            xdma(nc.sync, 1, 1)   # x1b
            nc.gpsimd.dma_start(bias_t[:], bias[:])
            for k in range(2, 8):
                wdma(nc.sync, k)
            for k in range(8, KCH):
                wdma(nc.gpsimd, k)

            # --- features per in-channel half h:
            # [x, q1=(x-k1)^2, c1=(x-k1)^3, r1c=relu(c1), r2c=relu((x-k2)^3)]
            # computed per free-dim half-chunk c for pipelining ---
            def ftile(nm, h):
                return fpool.tile([128, ROWS], fpr, tag=f"{nm}_{h}", name=f"{nm}_{h}")

            def stile(nm, h):
                return spool.tile([128, ROWS], fp, tag=f"{nm}_{h}", name=f"s{nm}_{h}")

            q1 = [ftile("q1", h) for h in range(2)]
            c1 = [ftile("c1", h) for h in range(2)]
            r1c = [ftile("r1c", h) for h in range(2)]
            r2c = [ftile("r2c", h) for h in range(2)]
            q2 = [stile("q2", h) for h in range(2)]
            c2 = [stile("c2", h) for h in range(2)]

            for h in range(2):
                for c in range(2):
                    sl = slice(c * HTOK, (c + 1) * HTOK)
                    x_ = xs[h][:, sl]
                    # ACT: squares directly from x (bias folds the shift)
                    nc.scalar.activation(q1[h][:, sl], x_, AF.Square, bias=negk1[:])
                    nc.scalar.activation(q2[h][:, sl], x_, AF.Square, bias=negk2[:])
                    # DVE: signed cubes via fused (x - k) * q
                    nc.vector.scalar_tensor_tensor(
                        c1[h][:, sl], x_, -k1, q1[h][:, sl], ALU.add, ALU.mult
                    )
                    nc.vector.scalar_tensor_tensor(
                        c2[h][:, sl], x_, -k2, q2[h][:, sl], ALU.add, ALU.mult
                    )
                    # truncate: relu(c) == relu(x-k)^3 elementwise
                    nc.scalar.activation(r1c[h][:, sl], c1[h][:, sl], AF.Relu)
                    nc.vector.tensor_scalar_max(r2c[h][:, sl], c2[h][:, sl], 0.0)

            feats = [xs, q1, c1, r1c, r2c]  # [f][h]

            ps = [
                [
                    ppool.tile(
                        [128, TOK], fp, tag=f"ps{oc}_{bt}", name=f"ps{oc}_{bt}"
                    )
                    for bt in range(BT)
                ]
                for oc in range(2)
            ]
            for ki in range(KCH):
                f, h = ki // 2, ki % 2
                rhs_full = feats[f][h]
                for oc in range(2):
                    lhsT = wt[ki][:, oc * 128:(oc + 1) * 128]
                    for bt in range(BT):
                        nc.tensor.matmul(
                            ps[oc][bt][:, :],
                            lhsT=lhsT,
                            rhs=rhs_full[:, bt * TOK:(bt + 1) * TOK],
                            start=(ki == 0),
                            stop=(ki == KCH - 1),
                        )
                        if ki == KCH - 1:
                            # evict this bank right after its last matmul;
                            # alternate ACT/DVE to split the tail work
                            ot = opool.tile(
                                [128, TOK], fp, tag=f"o{bt % 2}", name=f"o{oc}_{bt}"
                            )
                            if bt % 2 == 0:
                                nc.scalar.activation(
                                    ot[:],
                                    ps[oc][bt][:],
                                    AF.Identity,
                                    bias=bias_t[:, oc:oc + 1],
                                )
                            else:
                                nc.vector.tensor_scalar_add(
                                    ot[:], ps[oc][bt][:], bias_t[:, oc:oc + 1]
                                )
                            (nc.sync if bt % 2 == 0 else nc.scalar).dma_start(
                                y_t[
                                    oc * 128:(oc + 1) * 128,
                                    bt * TOK:(bt + 1) * TOK,
                                ],
                                ot[:],
                            )

    nc.compile()
    return nc


def _round_fp32r(a):
    """Round fp32 array to fp32r precision (8e11m: keep top 20 bits, RNE)."""
    u = np.ascontiguousarray(a, np.float32).view(np.uint32).astype(np.uint64)
    u = (u + 0x7FF + ((u >> 12) & 1)) & 0xFFFFF000
    return u.astype(np.uint32).view(np.float32)


def _prepare(x, spline_kernel, grid):
    A, k1, k2 = _solve_A(grid)
    W = np.asarray(spline_kernel, np.float64)  # [8, 256, 256]
    V = np.einsum("fk,kij->fij", A, W)  # [6, 256, 256]
    bias = V[0].sum(axis=0)  # [256]
    wf = _round_fp32r(
        V[1:].reshape(NF, 2, 128, COUT).reshape(KCH * 128, COUT).astype(np.float32)
    )
    bias_t = np.ascontiguousarray(bias.reshape(2, 128).T, dtype=np.float32)
    xf = np.asarray(x, np.float32).reshape(NCORES, ROWS, CIN)
    x_shards = _round_fp32r(xf.transpose(0, 2, 1))  # [8, 256, 2048]
    return x_shards, wf, bias_t, k1, k2


def _get_compiled(k1, k2):
    global _COMPILED
    if _COMPILED is None:
        _COMPILED = _build(k1, k2)
    return _COMPILED


_LDW_PATCHED = False


def _maybe_patch_ldw_opt():
    """Optionally flip walrus --enable-ldw-opt (dedupes repeated LDWEIGHTS)."""
    global _LDW_PATCHED
    import os

    if _LDW_PATCHED or os.environ.get("BSPLINE_LDW_OPT") == "0":
        return
    import concourse.bass_utils as bu

    orig = bu.run_command

    def patched(argv, **kw):
        argv = [
            a.replace("--enable-ldw-opt=false", "--enable-ldw-opt=true")
            for a in argv
        ]
        return orig(argv, **kw)

    bu.run_command = patched
    _LDW_PATCHED = True


def kernel(x, spline_kernel, grid, _trace=False):
    from concourse.bass_utils import run_bass_kernel_spmd

    _maybe_patch_ldw_opt()

    x_shards, wf, bias_t, k1, k2 = _prepare(x, spline_kernel, grid)
    nc = _get_compiled(k1, k2)
    in_maps = [
        {"x_t": x_shards[c], "w": wf, "bias": bias_t} for c in range(NCORES)
    ]
    res = run_bass_kernel_spmd(
        nc, in_maps, list(range(NCORES)), trace=_trace
    )
    y = np.stack([res.results[c]["y_t"].T for c in range(NCORES)])
    out = np.ascontiguousarray(y, dtype=np.float32).reshape(x.shape[0], x.shape[1], COUT)
    if _trace:
        kernel._last_results = res
    return out
